# revision 1
# baseline (speedup 1.0000x reference)
"""Trainium2 Bass kernel for nn_CfaModel (retrieval_knn).

Computes, for features [16, 3136, 1792], memory_bank [1792, 3136], radius [1]:
    distance[b,n,k] = ||f[b,n]||^2 + ||c[k]||^2 - 2 f.c
    vals = 6 smallest distances per (b,n)  (ascending)
    l_att = (1/NU) * mean(relu(vals[..., :3] - r^2))
    l_rep = (1/NU) * mean(relu(r^2 - vals[..., 3:] - ALPHA))
    out   = l_att + l_rep   (scalar, float32)

Strategy: data-parallel over batch across 8 NeuronCores (2 samples each).
Per core, TensorE streams fp8 DoubleRow matmuls computing
g' = 2 f.c - ||c||^2 directly: the -||c||^2 term is folded into the
matmul as a constant stationary weight pair (16, 1) at partition 0 of
the first contraction pass, matched with host-encoded rows
(-round(c_sq/16), -(c_sq - 16*round(c_sq/16))) in the moving operand
(two of the 1792 feature channels are dropped from the cross term to
make room; ||f||^2 and ||c||^2 keep all channels).  VectorE extracts
the top-8 of each 448-wide PSUM bank directly and merges them per row
tile; ||f||^2 is precomputed on host.  The epilogue turns the merged
top-6 into the two relu partial sums; the host sums the 8 cores'
[128, 2] outputs and applies the 1/(NU*count) scaling.
"""

import os
import threading

import numpy as np
import ml_dtypes

import concourse.bass as bass
import concourse.mybir as mybir
import concourse.tile as tile
from concourse import bacc
import concourse.bass_utils as bass_utils
from concourse.bass_utils import run_bass_kernel_spmd

# Problem constants (hardcoded per the harness contract).
B, HW, C, K = 16, 3136, 1792, 3136
NU, ALPHA = 0.001, 0.1
NCORES = 8
BPC = B // NCORES          # batches per core = 2
ROWS = BPC * HW            # rows per core = 6272
P = 128                    # partitions
NT = ROWS // P             # row tiles per core = 49
KC = C // P                # contraction chunks = 14
NQ = KC // 2               # DoubleRow passes = 7
CT = 7                     # column tiles
CW = K // CT               # column tile width = 448
NTP = (NT + 1) // 2        # fT DMA tile pairs (last one zero-padded) = 25

FP32 = mybir.dt.float32
BF16 = mybir.dt.bfloat16
FP8 = mybir.dt.float8e4
AF = mybir.ActivationFunctionType

NWARM = 52                 # PE warm-up matmuls issued during the input DMA
NT_A = NT - 2              # tiles covered by the early epilogue phase


def build_module(nt=NT):
    nc = bacc.Bacc(trn_type="TRN2", target_bir_lowering=False)

    # pre-transposed f (c on partitions); slot (chunk 0, part 0) holds the
    # constant 16 and (chunk 1, part 0) the constant 1 for the c_sq fold.
    # Pair-major layout so two row tiles move per DMA (3584B lines).
    fT_dram = nc.dram_tensor("fT", [NTP, P, 2, KC, P], FP8,
                             kind="ExternalInput")
    # 2*memory_bank, j-blocked: [CT, P(c%128), KC, CW]; rows (0, ci=0/1)
    # hold the -c_sq encoding
    m2_dram = nc.dram_tensor("m2", [CT, P, KC, CW], FP8, kind="ExternalInput")
    fsq_dram = nc.dram_tensor("fsq", [P, nt], FP32, kind="ExternalInput")
    # host-computed activation biases: [:, 0] = -r^2, [:, 1] = r^2 - ALPHA
    bias_dram = nc.dram_tensor("bias", [P, 2], FP32, kind="ExternalInput")
    out_dram = nc.dram_tensor("out", [P, 4], FP32, kind="ExternalOutput")

    with tile.TileContext(nc) as tc:
        with tc.tile_pool(name="singles", bufs=1) as singles:
            # ---- persistent tiles ----
            m2 = singles.tile([P, CT, KC, CW], FP8)
            fT = singles.tile([P, 2 * NTP, KC, P], FP8)  # all 49 row tiles (+pad)
            g_all = singles.tile([P, nt, CT, 8], FP32)   # per-bank top-8
            g8 = singles.tile([P, nt, 8], FP32)          # merged top-8 per row
            fsq = singles.tile([P, nt], FP32)            # ||f||^2 per row
            bias = singles.tile([P, 2], FP32)
            wz = singles.tile([P, P], BF16)              # zeros for PE warm-up
            u_all = singles.tile([P, nt, 8], FP32)       # u = g' - ||f||^2
            att_scr = singles.tile([P, nt, 3], FP32)
            rep_scr = singles.tile([P, nt, 3], FP32)
            outp = singles.tile([P, 4], FP32)

            nc.vector.memset(wz[:], 0.0)

            with (
                tc.tile_pool(name="mmp", bufs=4, space="PSUM") as mmp,
                tc.tile_pool(name="wmp", bufs=1, space="PSUM") as wmp,
            ):
                # DMA schedule (j-outer loop: only j0 and the fT tiles are
                # urgent; m2 blocks j1..j6 are consumed one 65us pass apart).
                # Per-ring bandwidth depends on line size, so fT tiles move
                # as PAIRS (3584B lines) alternating between the sync and
                # gpsimd rings; j0's halves ride sync + the fast-starting
                # scalar ring in parallel; later m2 blocks fill ring slack.
                # one urgent item per ring, in parallel; the scalar ring
                # starts most reliably, so it carries the gating fT pair
                nc.scalar.dma_start(fT[:, 0:2], fT_dram[0])
                nc.sync.dma_start(m2[:, 0, 0:8, :], m2_dram[0, :, 0:8, :])
                nc.gpsimd.dma_start(m2[:, 0, 8:, :], m2_dram[0, :, 8:, :])
                for b in range(1, NTP):
                    q_eng = nc.gpsimd if b % 2 == 1 else nc.sync
                    q_eng.dma_start(fT[:, 2 * b:2 * b + 2], fT_dram[b])
                    if b == 13:
                        nc.gpsimd.dma_start(m2[:, 3], m2_dram[3])
                    elif b == 17:
                        nc.gpsimd.dma_start(m2[:, 5], m2_dram[5])
                    elif b == 21:
                        nc.gpsimd.dma_start(m2[:, 6], m2_dram[6])
                nc.scalar.dma_start(m2[:, 1], m2_dram[1])
                nc.scalar.dma_start(m2[:, 2], m2_dram[2])
                nc.scalar.dma_start(m2[:, 4], m2_dram[4])
                nc.scalar.dma_start(fsq[:], fsq_dram[:])
                nc.scalar.dma_start(bias[:], bias_dram[:])

                # keep the PE busy (and the HAM clock gate open) while the
                # first fT/m2 blocks stream in; results are discarded
                warm_ps = wmp.tile([P, P], FP32, name="warm")
                for _ in range(NWARM):
                    nc.tensor.matmul(warm_ps[:], wz[:], wz[:],
                                     start=True, stop=True)

                def epilogue(lo, hi, col):
                    # u = g' - ||f||^2 = -distance for tiles [lo, hi)
                    nc.vector.tensor_sub(
                        u_all[:, lo:hi], g8[:, lo:hi],
                        fsq[:, lo:hi, None].to_broadcast([P, hi - lo, 8]),
                    )
                    # att = relu(distance - r^2) = relu(-u - r^2)
                    nc.scalar.activation(
                        att_scr[:, lo:hi], u_all[:, lo:hi, 0:3], AF.Relu,
                        bias=bias[:, 0:1], scale=-1.0,
                        accum_out=outp[:, col:col + 1],
                    )
                    # rep = relu(r^2 - distance - ALPHA) = relu(u + (r^2 - ALPHA))
                    nc.scalar.activation(
                        rep_scr[:, lo:hi], u_all[:, lo:hi, 3:6], AF.Relu,
                        bias=bias[:, 1:2], scale=1.0,
                        accum_out=outp[:, col + 1:col + 2],
                    )

                for j in range(CT):
                    for t in range(nt):
                        mm = mmp.tile([P, CW], FP32, name="acc")
                        for q in range(NQ):
                            nc.tensor.matmul(
                                mm[:],
                                fT[:, t, 2 * q:2 * q + 2, :],
                                m2[:, j, 2 * q:2 * q + 2, :],
                                start=(q == 0),
                                stop=(q == NQ - 1),
                                perf_mode=mybir.MatmulPerfMode.DoubleRow,
                            )
                        # top-8 of this 448-wide bank (descending)
                        nc.vector.max(out=g_all[:, t, j, :], in_=mm[:])
                        if j == CT - 1:
                            # all 7 candidate sets done -> global top-8
                            nc.vector.max(out=g8[:, t, :], in_=g_all[:, t])
                            if t == NT_A - 1:
                                # early epilogue overlaps the last tiles' MMs
                                epilogue(0, NT_A, 0)

            epilogue(NT_A, nt, 2)
            # scalar ring: its end-of-run drain comes later in the footer
            # chain, so the sync drain doesn't wait on this transfer
            nc.scalar.dma_start(out_dram[:], outp[:])

    nc.compile()
    return nc


_CACHE = {}
_LOCK = threading.Lock()
LAST_RESULT = None


def _get_module(nt=NT):
    with _LOCK:
        if nt not in _CACHE:
            _CACHE[nt] = build_module(nt)
        return _CACHE[nt]


def prep_inputs(features, memory_bank, radius):
    fp8 = ml_dtypes.float8_e4m3

    # fT: [core, pair, p_c (channel%128), t_in_pair, ci (chunk), r (row)],
    # fp8, pair-major so two row tiles ship per DMA.  Channels 0 and 128
    # (slots (ci=0,p=0), (ci=1,p=0)) are sacrificed to the c_sq constant
    # pair; f there is unused and overwritten by the constants 16, 1.
    fT = np.zeros((NCORES, 2 * NTP, P, KC, P), np.float32)
    fT[:, :NT] = features.reshape(NCORES, NT, P, KC, P).transpose(0, 1, 4, 3, 2)
    fT = fT.reshape(NCORES, NTP, 2, P, KC, P).transpose(0, 1, 3, 2, 4, 5).astype(fp8)
    fT[:, :, 0, :, 0, :] = fp8(16.0)
    fT[:, :, 0, :, 1, :] = fp8(1.0)

    # m2 = 2*memory_bank with C on partitions, quantized to fp8
    m2_base = (2.0 * memory_bank).reshape(KC, P, K).transpose(1, 0, 2)
    m2_q = m2_base.astype(fp8)

    # c_sq of the quantized bank (all 1792 channels), encoded as
    # 16*q1 + r in two fp8 rows replacing channels 0 and 128
    cq = m2_q.astype(np.float32) * 0.5
    c_sq = np.einsum('pck,pck->k', cq, cq, dtype=np.float32)
    q1 = np.round(c_sq / 16.0)
    res = c_sq - 16.0 * q1
    m2_q[0, 0, :] = (-q1).astype(fp8)
    m2_q[0, 1, :] = (-res).astype(fp8)

    # j-blocked layout: [CT, P, KC, CW]
    m2_blk = np.ascontiguousarray(
        m2_q.reshape(P, KC, CT, CW).transpose(2, 0, 1, 3)
    )

    # ||f||^2 per row, exact in fp32: [core, P, nt]
    fsq = np.einsum('bnc,bnc->bn', features, features, dtype=np.float32)
    fsq = np.ascontiguousarray(
        fsq.reshape(NCORES, NT, P).transpose(0, 2, 1)
    )

    # activation biases: [:, 0] = -r^2, [:, 1] = r^2 - ALPHA
    r2 = np.float32(radius.reshape(-1)[0]) ** 2
    bias = np.empty((P, 2), np.float32)
    bias[:, 0] = -r2
    bias[:, 1] = r2 - np.float32(ALPHA)
    return fT, m2_blk, fsq, bias


def kernel(features, memory_bank, radius):
    global LAST_RESULT
    features = np.asarray(features, dtype=np.float32)
    memory_bank = np.asarray(memory_bank, dtype=np.float32)
    radius = np.asarray(radius, dtype=np.float32)
    assert features.shape == (B, HW, C)
    assert memory_bank.shape == (C, K)

    nc = _get_module()

    # Shard: batch-parallel, 2 samples per core.  Low-precision cast on
    # host; the top-k / loss arithmetic stays fp32 on device.
    fT, m2_blk, fsq, bias = prep_inputs(features, memory_bank, radius)

    in_maps = [
        {"fT": fT[i], "m2": m2_blk, "fsq": fsq[i], "bias": bias}
        for i in range(NCORES)
    ]
    trace = bool(int(os.environ.get("KNN_TRACE", "0")))
    try:
        res = run_bass_kernel_spmd(
            nc, in_maps, core_ids=list(range(NCORES)), trace=trace
        )
    except ModuleNotFoundError:
        # axon NTFF profiling hook unavailable in this environment
        res = run_bass_kernel_spmd(
            nc, in_maps, core_ids=list(range(NCORES)), trace=False
        )
    LAST_RESULT = res

    parts = np.stack([r["out"] for r in res.results])   # [8, 128, 4]
    total = parts.sum(dtype=np.float64)                 # att + rep, both phases
    cnt = B * HW * 3
    loss = total / cnt / NU
    return np.float32(loss)

